# revision 33
# baseline (speedup 1.0000x reference)
"""Trainium2 Bass kernel for nn_DeepEC_KAN (DeepEC conv->maxpool->BN->LN->KAN x2).

Data parallel over batch (256 -> 32 per core on 8 cores). Per core:
  - host builds the full 6-tap im2col patch [126, 32, 1008] in bf16; streamed
    per-sample via HWDGE DMA (sync queue), prefetch depth 6. Weights ride the
    scalar-engine DMA queue so they don't delay the patch stream.
  - conv1d(K=4/8/16) = bf16 matmuls at column offsets 0/6/12 into the patch;
    per sample 12 matmuls (conv3 first, then conv2, conv1) into 3 PSUM tiles.
    bf16 streams the PE at 1 col/cycle @2.4GHz (f32r ran ~2x slower); a
    memset-fed dummy-matmul burst in the prologue pre-warms the PE clock
    (HAM starts throttled at 1.2GHz until ~3.4us of sustained activity).
  - maxpool: DVE reduce_max is the only max-capable engine path on TRN2
    (GpSimd has no TT-max ucode and no PSUM port; ACT accumulates sums only;
    tensor_tensor_reduce ucode is broken on this runtime; tensor_tensor_scan
    is recurrence-bound at 2cyc/step). So the pool is DVE-bound at ~1 elem/
    cycle: conv3 and conv2 reduce straight from PSUM in emission order
    (banks free progressively for the next sample's matmuls); conv1 is
    drained by the scalar engine (ACT copy PSUM->SBUF bf16, which frees its
    PSUM bank without DVE help) and reduced from SBUF. Steady-state sample
    period ~3.29us = the DVE reduce floor.
  - BN1..4 + conv bias folded into per-channel affine on host.
  - LayerNorm stats via ones-vector matmuls (cross-partition sums on PE).
  - KAN: silu via ACT Silu; cubic B-spline bases via the relu^3 cardinal
    form (fp32 intermediates - the 5-tap alternating-sign combination is
    catastrophically cancellative in bf16); contraction matmuls in bf16.
  - tail (LN+KAN) runs in two half-batches. Half 0's stages interleave
    between conv samples (two in-order streams - front LN/spline and back
    KAN-matmul - with at most one PE-heavy and one GpSimd-heavy stage per
    sample; the spline +-4/+6 combinations run as 2-op GpSimd forms there
    to keep the DVE free for the pool reduces). Half 1 is exposed after the
    loop; fp32 dummy matmuls anchored on its chain intermediates keep the
    PE p-state up through the serial elementwise stretches.
"""

import sys
import numpy as np

sys.path.insert(0, "/opt/trn_rl_repo")

import ml_dtypes  # noqa: E402

import concourse.bass as bass  # noqa: E402
import concourse.bacc as bacc  # noqa: E402
import concourse.tile as tile  # noqa: E402
from concourse import mybir  # noqa: E402
from concourse.bass import broadcast_tensor_aps  # noqa: E402
from concourse.bass_utils import run_bass_kernel_spmd  # noqa: E402

F32 = mybir.dt.float32
BF16 = mybir.dt.bfloat16
ALU = mybir.AluOpType
ACTF = mybir.ActivationFunctionType
AX = mybir.AxisListType

NCORES = 8
B = 256
BC = B // NCORES  # 32 samples per core
C = 21
L = 1000
LP = 1008
NH = BC // 2  # tail half-batch (16)
CONV_L = [997, 993, 985]
# emission order: conv3 groups, conv2 groups, conv1 (frees PSUM banks early)
GROUPS = [
    (252, 126, 0, 2, True, False),   # conv3 taps 0-5
    (378, 126, 6, 2, False, False),  # conv3 taps 6-11
    (504, 84, 12, 2, False, True),   # conv3 taps 12-15
    (84, 126, 0, 1, True, False),    # conv2 taps 0-5
    (210, 42, 6, 1, False, True),    # conv2 taps 6-7
    (0, 84, 0, 0, True, True),       # conv1 taps 0-3
]
WCONV_ROWS = 588
NW1 = 21
NW2 = 28
D1OUT = 512
D2OUT = 229
W2PAD = 256
PREFETCH = 6


def _build_program():
    nc = bacc.Bacc("TRN2", target_bir_lowering=False, debug=False,
                   num_devices=NCORES)
    patch_d = nc.dram_tensor("patch", [126, BC, LP], BF16,
                             kind="ExternalInput").ap()
    wconv = nc.dram_tensor("wconv", [WCONV_ROWS, 128], BF16,
                           kind="ExternalInput").ap()
    kconst = nc.dram_tensor("kconst", [128, 5, 96], F32,
                            kind="ExternalInput").ap()
    w1s_d = nc.dram_tensor("w1s", [128, NW1, D1OUT], BF16,
                           kind="ExternalInput").ap()
    w2s_d = nc.dram_tensor("w2s", [128, NW2, W2PAD], BF16,
                           kind="ExternalInput").ap()
    mtab_d = nc.dram_tensor("mtab", [128, 10], F32, kind="ExternalInput").ap()
    id32_d = nc.dram_tensor("id32", [32, 32], F32, kind="ExternalInput").ap()
    out_d = nc.dram_tensor("out", [BC, D2OUT], F32, kind="ExternalOutput").ap()

    with tile.TileContext(nc) as tc:
        with (
            tc.tile_pool(name="const", bufs=1) as const,
            tc.tile_pool(name="patches", bufs=PREFETCH + 1) as patches,
            tc.tile_pool(name="work", bufs=1) as work,
            tc.tile_pool(name="drain", bufs=2) as drain,
            tc.tile_pool(name="psconv", bufs=1, space="PSUM") as psconv,
            tc.tile_pool(name="pstail", bufs=1, space="PSUM") as pstail,
        ):
            # ---- constants (conv weights first; big tail weights streamed
            # in per-j slices on the in-order sync queue during the loop) ----
            wc_tiles = []
            for gi, (r0, nr, _off, _cj, _f, _l) in enumerate(GROUPS):
                wt = const.tile([128, 128], BF16, tag=f"wc{gi}", name=f"wc{gi}")
                wc_tiles.append(wt)
            kc = const.tile([128, 5, 96], F32, tag="kc", name="kc")
            mtab = const.tile([128, 10], F32, tag="mtab", name="mtab")
            w1s = const.tile([128, NW1, D1OUT], BF16, tag="w1s", name="w1s")
            w2s = const.tile([128, NW2, W2PAD], BF16, tag="w2s", name="w2s")
            id32 = const.tile([32, 32], F32, tag="id32", name="id32")
            ones = const.tile([128, 128], F32, tag="ones", name="ones")
            nc.vector.memset(ones, 1.0)
            wjobs = ([(w1s, w1s_d, j) for j in range(NW1)]
                     + [(w2s, w2s_d, j) for j in range(NW2)])
            # conv1 drain buffers (ACT copies PSUM here; DVE reduces from SBUF)
            c1sb_bufs = [drain.tile([128, 1000], BF16, tag="c1sb",
                                    name=f"c1sb{i}") for i in range(2)]

            mraw = work.tile([128, 3, BC], F32, tag="mraw", name="mraw")
            kc3 = kc.rearrange("p i (j b) -> p i j b", j=3)

            kc2d = kc.rearrange("p i w -> p (i w)")

            def make_front(b0, nb, hst, sfx, te, warm=None):
                """LN + spline1 stages for samples [b0, b0+nb).

                Writes sil / bs1 slices into the half-level tiles in hst
                (offset q0 = b0 - hst["b0"]).
                """
                W = 3 * nb
                q0 = b0 - hst["b0"]
                nbh = hst["nb"]
                st = {}

                def s0():
                    mrh = mraw[:, :, b0:b0 + nb]
                    kch = kc3[:, :, :, b0:b0 + nb]
                    t96 = work.tile([128, 3, nb], F32, tag=f"t96{sfx}",
                                    name=f"t96{sfx}")
                    te.tensor_add(t96, mrh, kch[:, 0])
                    h96 = work.tile([128, 3, nb], F32, tag=f"h96{sfx}",
                                    name=f"h96{sfx}")
                    nc.scalar.activation(out=h96, in_=t96, func=ACTF.Relu)
                    te.tensor_mul(h96, h96, kch[:, 1])
                    te.tensor_add(h96, h96, kch[:, 2])
                    st["h96"] = h96
                    if warm is not None:
                        nc.tensor.matmul(out=warm[0:16, 0:480],
                                         lhsT=h96[:, 0, 0:16], rhs=kc2d,
                                         start=True, stop=True)

                def s1():
                    h96 = st["h96"]
                    sq96 = work.tile([128, 3, nb], F32, tag=f"sq96{sfx}",
                                     name=f"sq96{sfx}")
                    nc.scalar.activation(out=sq96, in_=h96, func=ACTF.Square)
                    psLN = pstail.tile([1, 4 * W], F32, tag="small",
                                       name=f"psLN{sfx}")
                    nc.tensor.matmul(out=psLN[0:1, 0:W], lhsT=ones[:, 0:1],
                                     rhs=h96, start=True, stop=True)
                    nc.tensor.matmul(out=psLN[0:1, W:2 * W],
                                     lhsT=ones[:, 0:1], rhs=sq96,
                                     start=True, stop=True)
                    st["psLN"] = psLN

                def s2():
                    psLN = st["psLN"]
                    sums = work.tile([1, 2, nb], F32, tag=f"sums{sfx}",
                                     name=f"sums{sfx}")
                    psLNv = psLN[0:1, 0:2 * W].rearrange(
                        "p (x j b) -> p x b j", x=2, j=3)
                    nc.vector.reduce_sum(out=sums[0:1, 0], in_=psLNv[0:1, 0],
                                         axis=AX.X)
                    nc.vector.reduce_sum(out=sums[0:1, 1], in_=psLNv[0:1, 1],
                                         axis=AX.X)
                    muinv = work.tile([1, 2, nb], F32, tag=f"muinv{sfx}",
                                      name=f"muinv{sfx}")
                    nc.vector.tensor_scalar_mul(muinv[0:1, 0], sums[0:1, 0],
                                                1.0 / 384)
                    msq = work.tile([1, nb], F32, tag=f"msq{sfx}",
                                    name=f"msq{sfx}")
                    nc.vector.tensor_mul(msq, muinv[0:1, 0], muinv[0:1, 0])
                    var = work.tile([1, nb], F32, tag=f"var{sfx}",
                                    name=f"var{sfx}")
                    nc.vector.scalar_tensor_tensor(out=var, in0=sums[0:1, 1],
                                                   scalar=1.0 / 384, in1=msq,
                                                   op0=ALU.mult,
                                                   op1=ALU.subtract)
                    nc.vector.tensor_scalar_add(var, var, 1e-5)
                    sd = work.tile([1, nb], F32, tag=f"sd{sfx}",
                                   name=f"sd{sfx}")
                    nc.scalar.activation(out=sd, in_=var, func=ACTF.Sqrt,
                                         bias=0.0)
                    st["sd"] = sd
                    st["muinv"] = muinv

                def s2b():
                    # isolated: waits on ACT Sqrt (+table load); keeping it in
                    # its own stage stops it stalling conv reduces behind it
                    nc.vector.reciprocal(st["muinv"][0:1, 1], st["sd"])

                def s3():
                    psB = pstail.tile([128, 2, nb], F32, tag="small",
                                      name=f"psB{sfx}")
                    nc.tensor.matmul(out=psB, lhsT=ones[0:1, :],
                                     rhs=st["muinv"][0:1], start=True,
                                     stop=True)
                    muinvB = work.tile([128, 2, nb], F32, tag=f"muinvB{sfx}",
                                       name=f"muinvB{sfx}")
                    nc.scalar.copy(out=muinvB, in_=psB)
                    st["muinvB"] = muinvB

                def s4():
                    h96, muinvB = st["h96"], st["muinvB"]
                    kch = kc3[:, :, :, b0:b0 + nb]
                    hn = work.tile([128, 3, nb], F32, tag=f"hn{sfx}",
                                   name=f"hn{sfx}")
                    for j in range(3):
                        te.tensor_sub(hn[:, j], h96[:, j], muinvB[:, 0])
                        te.tensor_mul(hn[:, j], hn[:, j], muinvB[:, 1])
                    te.tensor_mul(hn, hn, kch[:, 3])
                    te.tensor_add(hn, hn, kch[:, 4])
                    st["hn"] = hn
                    if warm is not None:
                        nc.tensor.matmul(out=warm[0:16, 0:480],
                                         lhsT=hn[:, 0, 0:16], rhs=kc2d,
                                         start=True, stop=True)

                def s5():
                    sil = hst["sil"]
                    nc.scalar.activation(out=sil[:, :, q0:q0 + nb],
                                         in_=st["hn"], func=ACTF.Silu)

                def f1():
                    hn2d = st["hn"].rearrange("p j b -> p (j b)")
                    x3 = hn2d.rearrange("p (m w) -> p m w", m=1)
                    m3 = mtab.rearrange("p (m w) -> p m w", w=1)
                    bx, bm = broadcast_tensor_aps(x3, m3)
                    d = work.tile([128, 10, W], F32, tag=f"sp_d{W}",
                                  name=f"sp1{sfx}_d")
                    te.tensor_tensor(out=d, in0=bx, in1=bm, op=ALU.subtract)
                    st["d"] = d

                def f2a():
                    d = st["d"]
                    v = work.tile([128, 10, W], F32, tag=f"sp_v{W}",
                                  name=f"sp1{sfx}_v")
                    nc.scalar.activation(out=v, in_=d, func=ACTF.Relu)
                    v2 = work.tile([128, 10, W], F32, tag=f"sp_v2{W}",
                                   name=f"sp1{sfx}_v2")
                    nc.scalar.activation(out=v2, in_=v, func=ACTF.Square)
                    r = work.tile([128, 10, W], F32, tag=f"sp_r{W}",
                                  name=f"sp1{sfx}_r")
                    te.tensor_mul(r, v2, v)
                    st["r"] = r
                    if warm is not None:
                        nc.tensor.matmul(out=warm[0:16, 0:480],
                                         lhsT=r[:, 0, 0:16], rhs=kc2d,
                                         start=True, stop=True)

                def f2b1():
                    r = st["r"]
                    a = work.tile([128, 6, W], F32, tag=f"sp_s1{W}",
                                  name=f"sp1{sfx}_a")
                    te.tensor_add(a, r[:, 0:6], r[:, 4:10])
                    st["a"] = a

                def f2b2():
                    r = st["r"]
                    bsum = work.tile([128, 6, W], F32, tag=f"sp_s2{W}",
                                     name=f"sp1{sfx}_b")
                    te.tensor_add(bsum, r[:, 1:7], r[:, 3:9])
                    st["bsum"] = bsum

                gs = te is nc.gpsimd
                bs_re = "p g (j b) -> p g j b"

                def f2c1():
                    a, bsum = st["a"], st["bsum"]
                    if warm is not None:
                        nc.tensor.matmul(out=warm[0:16, 0:480],
                                         lhsT=bsum[:, 0, 0:16], rhs=kc2d,
                                         start=True, stop=True)
                    t1 = work.tile([128, 6, W], F32, tag=f"sp_s12{W}",
                                   name=f"sp1{sfx}_t1")
                    if gs:
                        tmp = work.tile([128, 6, W], F32, tag=f"sp_tm{W}",
                                        name=f"sp1{sfx}_tm")
                        nc.gpsimd.tensor_scalar_mul(tmp, bsum, -4.0)
                        nc.gpsimd.tensor_add(t1, tmp, a)
                    else:
                        nc.vector.scalar_tensor_tensor(
                            out=t1, in0=bsum, scalar=-4.0, in1=a,
                            op0=ALU.mult, op1=ALU.add)
                    st["t1"] = t1

                def f2c2():
                    r, t1 = st["r"], st["t1"]
                    bs_view = hst["bs1"][:, :, :, q0:q0 + nb]
                    if gs:
                        tmp = work.tile([128, 6, W], F32, tag=f"sp_tm{W}",
                                        name=f"sp1{sfx}_t2")
                        nc.gpsimd.tensor_scalar_mul(tmp, r[:, 2:8], 6.0)
                        nc.vector.tensor_add(
                            bs_view, tmp.rearrange(bs_re, j=3),
                            t1.rearrange(bs_re, j=3))
                    else:
                        nc.vector.scalar_tensor_tensor(
                            out=bs_view,
                            in0=r[:, 2:8].rearrange(bs_re, j=3),
                            scalar=6.0,
                            in1=t1.rearrange(bs_re, j=3),
                            op0=ALU.mult, op1=ALU.add)

                L, G, P = "lt", "gs", "pe"
                return [(L, s0), (L, s1), (L, s2), (L, s2b), (L, s3),
                        (L, s4), (L, s5), (G, f1), (G, f2a), (G, f2b1),
                        (G, f2b2), (G, f2c1), (G, f2c2)]

            def make_back(hst, sfx, te, warm=None):
                """KAN matmuls + layer-2 for the half described by hst."""
                nb = hst["nb"]
                b0 = hst["b0"]
                st = {}

                def s7a():
                    sil = hst["sil"].rearrange("p j b -> p (j b)")
                    psK1 = pstail.tile([nb, D1OUT], F32, tag="big",
                                       name=f"psK1{sfx}")
                    for j in range(3):
                        nc.tensor.matmul(out=psK1,
                                         lhsT=sil[:, j * nb:(j + 1) * nb],
                                         rhs=w1s[:, j], start=(j == 0),
                                         stop=False)
                    st["psK1"] = psK1

                def mk_s7(j):
                    def s7x():
                        bs1 = hst["bs1"].rearrange("p g j b -> p g (j b)")
                        psK1 = st["psK1"]
                        for g in range(6):
                            nc.tensor.matmul(
                                out=psK1, lhsT=bs1[:, g, j * nb:(j + 1) * nb],
                                rhs=w1s[:, 3 + j * 6 + g],
                                start=False, stop=False)
                    return s7x

                def s8():
                    bs1 = hst["bs1"].rearrange("p g j b -> p g (j b)")
                    psK1 = st["psK1"]
                    for g in range(6):
                        nc.tensor.matmul(
                            out=psK1,
                            lhsT=bs1[:, g, 2 * nb:3 * nb],
                            rhs=w1s[:, 15 + g],
                            start=False, stop=(g == 5))
                    h2s = work.tile([nb, D1OUT], F32, tag=f"h2s{sfx}",
                                    name=f"h2s{sfx}")
                    nc.scalar.copy(out=h2s, in_=psK1)
                    st["h2s"] = h2s

                def s9():
                    h2s = st["h2s"]
                    psT = pstail.tile([128, 4 * nb], F32, tag="big",
                                      name=f"psT{sfx}")
                    for j in range(4):
                        nc.tensor.transpose(out=psT[:, j * nb:(j + 1) * nb],
                                            in_=h2s[:, j * 128:(j + 1) * 128],
                                            identity=id32[0:nb, 0:nb])
                    h2T = work.tile([128, 4 * nb], F32, tag=f"h2T{sfx}",
                                    name=f"h2T{sfx}")
                    nc.scalar.copy(out=h2T, in_=psT)
                    sil2 = work.tile([128, 4 * nb], BF16, tag=f"sil2{sfx}",
                                     name=f"sil2{sfx}")
                    nc.scalar.activation(out=sil2, in_=h2T, func=ACTF.Silu)
                    st["h2T"] = h2T
                    st["sil2"] = sil2

                def g1():
                    W = 4 * nb
                    x3 = st["h2T"].rearrange("p (m w) -> p m w", m=1)
                    m3 = mtab.rearrange("p (m w) -> p m w", w=1)
                    bx, bm = broadcast_tensor_aps(x3, m3)
                    d = work.tile([128, 10, W], F32, tag=f"sp_d{W}",
                                  name=f"sp2{sfx}_d")
                    te.tensor_tensor(out=d, in0=bx, in1=bm, op=ALU.subtract)
                    st["d2"] = d
                    if warm is not None:
                        nc.tensor.matmul(out=warm[0:16, 0:480],
                                         lhsT=d[:, 0, 0:16], rhs=kc2d,
                                         start=True, stop=True)

                def g2a():
                    W = 4 * nb
                    d = st["d2"]
                    v = work.tile([128, 10, W], F32, tag=f"sp_v{W}",
                                  name=f"sp2{sfx}_v")
                    nc.scalar.activation(out=v, in_=d, func=ACTF.Relu)
                    v2 = work.tile([128, 10, W], F32, tag=f"sp_v2{W}",
                                   name=f"sp2{sfx}_v2")
                    nc.scalar.activation(out=v2, in_=v, func=ACTF.Square)
                    r = work.tile([128, 10, W], F32, tag=f"sp_r{W}",
                                  name=f"sp2{sfx}_r")
                    te.tensor_mul(r, v2, v)
                    st["r2"] = r

                def g2b1():
                    W = 4 * nb
                    r = st["r2"]
                    a = work.tile([128, 6, W], F32, tag=f"sp_s1{W}",
                                  name=f"sp2{sfx}_a")
                    te.tensor_add(a, r[:, 0:6], r[:, 4:10])
                    st["a2"] = a
                    if warm is not None:
                        nc.tensor.matmul(out=warm[0:16, 0:480],
                                         lhsT=a[:, 0, 0:16], rhs=kc2d,
                                         start=True, stop=True)

                def g2b2():
                    W = 4 * nb
                    r = st["r2"]
                    bsum = work.tile([128, 6, W], F32, tag=f"sp_s2{W}",
                                     name=f"sp2{sfx}_b")
                    te.tensor_add(bsum, r[:, 1:7], r[:, 3:9])
                    st["bsum2"] = bsum

                gs = te is nc.gpsimd

                def g2c1():
                    W = 4 * nb
                    a, bsum = st["a2"], st["bsum2"]
                    t1 = work.tile([128, 6, W], F32, tag=f"sp_s12{W}",
                                   name=f"sp2{sfx}_t1")
                    if gs:
                        tmp = work.tile([128, 6, W], F32, tag=f"sp_tm{W}",
                                        name=f"sp2{sfx}_tm")
                        nc.gpsimd.tensor_scalar_mul(tmp, bsum, -4.0)
                        nc.gpsimd.tensor_add(t1, tmp, a)
                    else:
                        nc.vector.scalar_tensor_tensor(
                            out=t1, in0=bsum, scalar=-4.0, in1=a,
                            op0=ALU.mult, op1=ALU.add)
                    st["t12"] = t1

                def g2c2():
                    W = 4 * nb
                    r, t1 = st["r2"], st["t12"]
                    bs = work.tile([128, 6, W], BF16, tag=f"sp_bs{W}",
                                   name=f"sp2{sfx}_bs")
                    if gs:
                        tmp = work.tile([128, 6, W], F32, tag=f"sp_tm{W}",
                                        name=f"sp2{sfx}_t2")
                        nc.gpsimd.tensor_scalar_mul(tmp, r[:, 2:8], 6.0)
                        nc.vector.tensor_add(bs, tmp, t1)
                    else:
                        nc.vector.scalar_tensor_tensor(
                            out=bs, in0=r[:, 2:8], scalar=6.0, in1=t1,
                            op0=ALU.mult, op1=ALU.add)
                    st["bs2"] = bs

                def s11a():
                    psK2 = pstail.tile([nb, W2PAD], F32, tag="big",
                                       name=f"psK2{sfx}")
                    for j in range(4):
                        nc.tensor.matmul(out=psK2,
                                         lhsT=st["sil2"][:, j * nb:(j + 1) * nb],
                                         rhs=w2s[:, j], start=(j == 0),
                                         stop=False)
                    st["psK2"] = psK2

                def mk_s11(j):
                    def s11x():
                        psK2 = st["psK2"]
                        for g in range(6):
                            nc.tensor.matmul(
                                out=psK2, lhsT=st["bs2"][:, g, j * nb:(j + 1) * nb],
                                rhs=w2s[:, 4 + j * 6 + g],
                                start=False, stop=False)
                    return s11x

                def s12():
                    psK2 = st["psK2"]
                    for g in range(6):
                        nc.tensor.matmul(
                            out=psK2,
                            lhsT=st["bs2"][:, g, 3 * nb:4 * nb],
                            rhs=w2s[:, 22 + g],
                            start=False, stop=(g == 5))
                    outS = work.tile([nb, D2OUT], F32, tag=f"outS{sfx}",
                                     name=f"outS{sfx}")
                    nc.scalar.copy(out=outS, in_=psK2[:, 0:D2OUT])
                    nc.sync.dma_start(out=out_d[b0:b0 + nb], in_=outS)

                L, G, P = "lt", "gs", "pe"
                return [(P, s7a), (P, mk_s7(0)), (P, mk_s7(1)),
                        (P, s8), (L, s9), (G, g1), (G, g2a),
                        (G, g2b1), (G, g2b2), (G, g2c1), (G, g2c2),
                        (P, s11a), (P, mk_s11(0)), (P, mk_s11(1)),
                        (P, mk_s11(2)), (P, s12)]

            def make_half_tiles(b0, nbh, hx):
                sil = work.tile([128, 3, nbh], BF16, tag=f"silh{hx}",
                                name=f"silh{hx}")
                bs1 = work.tile([128, 6, 3, nbh], BF16, tag=f"bs1h{hx}",
                                name=f"bs1h{hx}")
                return {"b0": b0, "nb": nbh, "sil": sil, "bs1": bs1}

            # ---- conv phase with tail stages interleaved ----
            # half-0 (samples 0-15): LN+spline front and KAN back interleave
            # between conv samples from sample 17
            h0 = make_half_tiles(0, NH, 0)
            h1 = make_half_tiles(NH, NH, 1)
            fstages = make_front(0, NH, h0, "h0", nc.gpsimd)
            bstages = make_back(h0, "h0", nc.gpsimd)
            # back-stage i needs front progress >= bpreq[i] (s7a needs sil
            # from s5; the spline-j matmuls need bs1 complete from f2c2)
            bpreq = [7, 13, 13, 13] + [13] * (len(bstages) - 4)
            fi = bi = 0
            tile_of = {}

            def load(b):
                pt = patches.tile([128, LP], BF16, tag="pt", name=f"pt{b}")
                nc.sync.dma_start(out=pt[0:126], in_=patch_d[:, b, :])
                tile_of[b] = pt

            load(0)
            for gi, (r0, nr, _off, _cj, _f, _l) in enumerate(GROUPS):
                nc.scalar.dma_start(out=wc_tiles[gi][0:nr, :],
                                    in_=wconv[r0:r0 + nr, :])
            nc.scalar.dma_start(out=kc, in_=kconst)
            nc.scalar.dma_start(out=mtab, in_=mtab_d)
            nc.scalar.dma_start(out=id32, in_=id32_d)
            for b in range(1, PREFETCH):
                load(b)

            wz = const.tile([128, 512], BF16, tag="wz", name="wz")
            nc.vector.memset(wz, 0.5)
            pswarm = pstail.tile([16, D1OUT], F32, tag="big", name="pswarm")
            for _ in range(10):
                nc.tensor.matmul(out=pswarm, lhsT=wz[:, 0:16],
                                 rhs=wz, start=True, stop=True)

            last_pt = None
            for b in range(BC):
                if b + PREFETCH < BC:
                    load(b + PREFETCH)
                if b >= 4:
                    for _ in range(5):
                        if wjobs:
                            wt, wd, j = wjobs.pop(0)
                            nc.sync.dma_start(out=wt[:, j], in_=wd[:, j])
                pt = tile_of.pop(b)
                last_pt = pt
                pc = [psconv.tile([128, 1024], F32, tag=f"pc{j}",
                                  name=f"pc{j}") for j in range(3)]
                for gi, (r0, nr, off, cj, first, last) in enumerate(GROUPS):
                    lcj = CONV_L[cj] + (CONV_L[cj] & 1)
                    for (n0, n1) in ((0, 512), (512, lcj)):
                        nc.tensor.matmul(
                            out=pc[cj][:, n0:n1],
                            lhsT=wc_tiles[gi][0:nr, :],
                            rhs=pt[0:nr, off + n0: off + n1],
                            start=first, stop=last,
                        )
                # maxpool: DVE reduces c3 then c2 from PSUM (in emission
                # order, so banks free progressively for sample b+1); conv1
                # drains through ACT to SBUF (frees pc0 without DVE) and a
                # final DVE reduce runs from SBUF.
                c1sb = c1sb_bufs[b % 2]
                nc.vector.reduce_max(out=mraw[:, 2, b:b + 1],
                                     in_=pc[2][:, 0:CONV_L[2]], axis=AX.X)
                nc.scalar.copy(out=c1sb[:, 0:CONV_L[0]],
                               in_=pc[0][:, 0:CONV_L[0]])
                nc.vector.reduce_max(out=mraw[:, 1, b:b + 1],
                                     in_=pc[1][:, 0:CONV_L[1]], axis=AX.X)
                # conv1's SBUF reduce runs one sample late: by then the ACT
                # copy is certainly finished, so it never bubbles the DVE
                if b > 0:
                    nc.vector.reduce_max(out=mraw[:, 0, b - 1:b],
                                         in_=c1sb_bufs[(b - 1) % 2][:, 0:CONV_L[0]],
                                         axis=AX.X)
                if b == BC - 1:
                    nc.vector.reduce_max(out=mraw[:, 0, b:b + 1],
                                         in_=c1sb[:, 0:CONV_L[0]], axis=AX.X)
                # interleave tail stages: front and back advance as two
                # independent in-order streams, at most one heavy stage per
                # engine class per sample (PE matmul bursts vs GpSimd spline)
                if b >= 16:
                    used = set()
                    taken = 0
                    while taken < 3:
                        prog = False
                        if fi < len(fstages):
                            cls = fstages[fi][0]
                            if cls == "lt" or cls not in used:
                                used.add(cls)
                                fstages[fi][1]()
                                fi += 1
                                taken += 1
                                prog = True
                        if (taken < 3 and bi < len(bstages)
                                and bpreq[bi] <= fi):
                            cls = bstages[bi][0]
                            if cls == "lt" or cls not in used:
                                used.add(cls)
                                bstages[bi][1]()
                                bi += 1
                                taken += 1
                                prog = True
                        if not prog:
                            break
            while fi < len(fstages):
                fstages[fi][1]()
                fi += 1
            while bi < len(bstages):
                bstages[bi][1]()
                bi += 1
            # exposed tail: half-1 front + back; anchored fp32 dummy matmuls
            # (fed by chain intermediates) keep the PE p-state at full clock
            # through the serial elementwise stretches so K1/K2 run warm
            pcw = psconv.tile([128, 1024], F32, tag="pc0", name="pcw")
            for _cls, s in (make_front(NH, NH, h1, "h1", nc.vector, warm=pcw)
                            + make_back(h1, "h1", nc.vector, warm=pcw)):
                s()
    nc.compile()
    return nc


def _host_prep(inputs):
    f = np.float32
    bf = ml_dtypes.bfloat16
    x = np.asarray(inputs["x"], f)
    xT = np.ascontiguousarray(x.transpose(0, 2, 1))  # [B, 21, 1000]
    xTpad = np.zeros((B, C, LP + 5), f)
    xTpad[:, :, :L] = xT
    # full 6-tap patch: patch[s*21+c, b, col] = x[b, c, col+s]
    pat = np.empty((6, C, B, LP), f)
    for s in range(6):
        pat[s] = xTpad[:, :, s:s + LP].transpose(1, 0, 2)
    pat = pat.reshape(126, B, LP).astype(bf)
    patches = [np.ascontiguousarray(pat[:, i * BC:(i + 1) * BC, :])
               for i in range(NCORES)]

    def chunks(w, taps):
        return [np.ascontiguousarray(
            np.asarray(w, f)[:, :, t0:t1].transpose(2, 1, 0).reshape((t1 - t0) * C, 128))
            for t0, t1 in taps]

    wconv = np.concatenate(
        chunks(inputs["conv1_w"], [(0, 4)])
        + chunks(inputs["conv2_w"], [(0, 6), (6, 8)])
        + chunks(inputs["conv3_w"], [(0, 6), (6, 12), (12, 16)]), 0).astype(bf)

    def fold(p):
        g, bb, m, v = (np.asarray(inputs[p + s], f) for s in ("_g", "_b", "_m", "_v"))
        s = g / np.sqrt(v + 1e-5)
        return s, bb - m * s

    s1, t1 = fold("bn1")
    s2, t2 = fold("bn2")
    s3, t3 = fold("bn3")
    s4, t4 = fold("bn4")
    Sall = np.concatenate([s1, s2, s3]) * s4
    Tall = np.concatenate([t1, t2, t3]) * s4 + t4
    cb = np.concatenate([np.asarray(inputs["conv1_b"], f),
                         np.asarray(inputs["conv2_b"], f),
                         np.asarray(inputs["conv3_b"], f)])

    def expand(v):
        return np.repeat(np.asarray(v, f).reshape(3, 128).T[:, :, None], BC, 2)

    kconst = np.stack([expand(cb), expand(Sall), expand(Tall),
                       expand(np.asarray(inputs["ln_g"], f)),
                       expand(np.asarray(inputs["ln_b"], f))], 1)
    kconst = np.ascontiguousarray(kconst.reshape(128, 5, 96))

    # spline scale: bases from relu^3 form come out as (6*3.375)*B_g when
    # v = relu(x - (m-4.5)/1.5); fold 1/(6/3.375) into the spline weights
    spl_scale = 3.375 / 6.0
    bw1 = np.asarray(inputs["base_w1"], f)
    sw1 = np.asarray(inputs["spline_w1"], f) * spl_scale
    w1s = np.empty((128, NW1, D1OUT), f)
    for j in range(3):
        w1s[:, j, :] = bw1[:, j * 128:(j + 1) * 128].T
        for g in range(6):
            w1s[:, 3 + j * 6 + g, :] = sw1[:, j * 128:(j + 1) * 128, g].T
    bw2 = np.asarray(inputs["base_w2"], f)
    sw2 = np.asarray(inputs["spline_w2"], f) * spl_scale
    w2s = np.zeros((128, NW2, W2PAD), f)
    for j in range(4):
        w2s[:, j, :D2OUT] = bw2[:, j * 128:(j + 1) * 128].T
        for g in range(6):
            w2s[:, 4 + j * 6 + g, :D2OUT] = sw2[:, j * 128:(j + 1) * 128, g].T

    mtab = np.tile(((np.arange(10, dtype=f) - 4.5) / 1.5), (128, 1))

    shared = {
        "wconv": np.ascontiguousarray(wconv),
        "kconst": kconst,
        "w1s": np.ascontiguousarray(w1s.astype(bf)),
        "w2s": np.ascontiguousarray(w2s.astype(bf)),
        "mtab": np.ascontiguousarray(mtab),
        "id32": np.eye(32, dtype=f),
    }
    return shared, patches


_NC_CACHE = None


def _get_nc():
    global _NC_CACHE
    if _NC_CACHE is None:
        _NC_CACHE = _build_program()
    return _NC_CACHE


def make_in_maps(inputs):
    shared, patches = _host_prep(inputs)
    return [{**shared, "patch": patches[i]} for i in range(NCORES)]


def kernel(**inputs):
    nc = _get_nc()
    in_maps = make_in_maps(inputs)
    res = run_bass_kernel_spmd(nc, in_maps, list(range(NCORES)))
    return np.concatenate([res.results[i]["out"] for i in range(NCORES)], 0)


# revision 34
# speedup vs baseline: 1.0210x; 1.0210x over previous
"""Trainium2 Bass kernel for nn_DeepEC_KAN (DeepEC conv->maxpool->BN->LN->KAN x2).

Data parallel over batch (256 -> 32 per core on 8 cores). Per core:
  - host builds the full 6-tap im2col patch [126, 32, 1008] in bf16; streamed
    per-sample via HWDGE DMA (sync queue), prefetch depth 6. Weights ride the
    scalar-engine DMA queue so they don't delay the patch stream.
  - conv1d(K=4/8/16) = bf16 matmuls at column offsets 0/6/12 into the patch;
    per sample 12 matmuls (conv3 first, then conv2, conv1) into 3 PSUM tiles.
    bf16 streams the PE at 1 col/cycle @2.4GHz (f32r ran ~2x slower); a
    memset-fed dummy-matmul burst in the prologue pre-warms the PE clock
    (HAM starts throttled at 1.2GHz until ~3.4us of sustained activity).
  - maxpool: DVE reduce_max is the only max-capable engine path on TRN2
    (GpSimd has no TT-max ucode and no PSUM port; ACT accumulates sums only;
    tensor_tensor_reduce ucode is broken on this runtime; tensor_tensor_scan
    is recurrence-bound at 2cyc/step). So the pool is DVE-bound at ~1 elem/
    cycle: conv3 and conv2 reduce straight from PSUM in emission order
    (banks free progressively for the next sample's matmuls); conv1 is
    drained by the scalar engine (ACT copy PSUM->SBUF bf16, which frees its
    PSUM bank without DVE help) and reduced from SBUF. Steady-state sample
    period ~3.29us = the DVE reduce floor.
  - BN1..4 + conv bias folded into per-channel affine on host.
  - LayerNorm stats via ones-vector matmuls (cross-partition sums on PE).
  - KAN: silu via ACT Silu; cubic B-spline bases via the relu^3 cardinal
    form (fp32 intermediates - the 5-tap alternating-sign combination is
    catastrophically cancellative in bf16); contraction matmuls in bf16.
  - tail (LN+KAN) runs in two half-batches. Half 0's stages interleave
    between conv samples (two in-order streams - front LN/spline and back
    KAN-matmul - with at most one PE-heavy and one GpSimd-heavy stage per
    sample; the spline +-4/+6 combinations run as 2-op GpSimd forms there
    to keep the DVE free for the pool reduces). Half 1 is exposed after the
    loop; fp32 dummy matmuls anchored on its chain intermediates keep the
    PE p-state up through the serial elementwise stretches.
"""

import sys
import numpy as np

sys.path.insert(0, "/opt/trn_rl_repo")

import ml_dtypes  # noqa: E402

import concourse.bass as bass  # noqa: E402
import concourse.bacc as bacc  # noqa: E402
import concourse.tile as tile  # noqa: E402
from concourse import mybir  # noqa: E402
from concourse.bass import broadcast_tensor_aps  # noqa: E402
from concourse.bass_utils import run_bass_kernel_spmd  # noqa: E402

F32 = mybir.dt.float32
BF16 = mybir.dt.bfloat16
ALU = mybir.AluOpType
ACTF = mybir.ActivationFunctionType
AX = mybir.AxisListType

NCORES = 8
B = 256
BC = B // NCORES  # 32 samples per core
C = 21
L = 1000
LP = 1008
NH = BC // 2  # tail half-batch (16)
CONV_L = [997, 993, 985]
# emission order: conv3 groups, conv2 groups, conv1 (frees PSUM banks early)
GROUPS = [
    (252, 126, 0, 2, True, False),   # conv3 taps 0-5
    (378, 126, 6, 2, False, False),  # conv3 taps 6-11
    (504, 84, 12, 2, False, True),   # conv3 taps 12-15
    (84, 126, 0, 1, True, False),    # conv2 taps 0-5
    (210, 42, 6, 1, False, True),    # conv2 taps 6-7
    (0, 84, 0, 0, True, True),       # conv1 taps 0-3
]
WCONV_ROWS = 588
NW1 = 21
NW2 = 28
D1OUT = 512
D2OUT = 229
W2PAD = 256
PREFETCH = 6


def _build_program():
    nc = bacc.Bacc("TRN2", target_bir_lowering=False, debug=False,
                   num_devices=NCORES)
    patch_d = nc.dram_tensor("patch", [126, BC, LP], BF16,
                             kind="ExternalInput").ap()
    wconv = nc.dram_tensor("wconv", [WCONV_ROWS, 128], BF16,
                           kind="ExternalInput").ap()
    kconst = nc.dram_tensor("kconst", [128, 5, 96], F32,
                            kind="ExternalInput").ap()
    w1s_d = nc.dram_tensor("w1s", [128, NW1, D1OUT], BF16,
                           kind="ExternalInput").ap()
    w2s_d = nc.dram_tensor("w2s", [128, NW2, W2PAD], BF16,
                           kind="ExternalInput").ap()
    mtab_d = nc.dram_tensor("mtab", [128, 10], F32, kind="ExternalInput").ap()
    id32_d = nc.dram_tensor("id32", [32, 32], F32, kind="ExternalInput").ap()
    out_d = nc.dram_tensor("out", [BC, D2OUT], F32, kind="ExternalOutput").ap()

    with tile.TileContext(nc) as tc:
        with (
            tc.tile_pool(name="const", bufs=1) as const,
            tc.tile_pool(name="patches", bufs=PREFETCH + 1) as patches,
            tc.tile_pool(name="work", bufs=1) as work,
            tc.tile_pool(name="drain", bufs=2) as drain,
            tc.tile_pool(name="psconv", bufs=1, space="PSUM") as psconv,
            tc.tile_pool(name="pstail", bufs=1, space="PSUM") as pstail,
        ):
            # ---- constants (conv weights first; big tail weights streamed
            # in per-j slices on the in-order sync queue during the loop) ----
            wc_tiles = []
            for gi, (r0, nr, _off, _cj, _f, _l) in enumerate(GROUPS):
                wt = const.tile([128, 128], BF16, tag=f"wc{gi}", name=f"wc{gi}")
                wc_tiles.append(wt)
            kc = const.tile([128, 5, 96], F32, tag="kc", name="kc")
            mtab = const.tile([128, 10], F32, tag="mtab", name="mtab")
            w1s = const.tile([128, NW1, D1OUT], BF16, tag="w1s", name="w1s")
            w2s = const.tile([128, NW2, W2PAD], BF16, tag="w2s", name="w2s")
            id32 = const.tile([32, 32], F32, tag="id32", name="id32")
            ones = const.tile([128, 128], F32, tag="ones", name="ones")
            nc.vector.memset(ones, 1.0)
            wjobs = ([(w1s, w1s_d, j) for j in range(NW1)]
                     + [(w2s, w2s_d, j) for j in range(NW2)])
            # conv1 drain buffers (ACT copies PSUM here; DVE reduces from SBUF)
            c1sb_bufs = [drain.tile([128, 1000], BF16, tag="c1sb",
                                    name=f"c1sb{i}") for i in range(2)]

            mraw = work.tile([128, 3, BC], F32, tag="mraw", name="mraw")
            kc3 = kc.rearrange("p i (j b) -> p i j b", j=3)

            kc2d = kc.rearrange("p i w -> p (i w)")

            def make_front(b0, nb, hst, sfx, te, warm=None):
                """LN + spline1 stages for samples [b0, b0+nb).

                Writes sil / bs1 slices into the half-level tiles in hst
                (offset q0 = b0 - hst["b0"]).
                """
                W = 3 * nb
                q0 = b0 - hst["b0"]
                nbh = hst["nb"]
                st = {}

                def s0():
                    mrh = mraw[:, :, b0:b0 + nb]
                    kch = kc3[:, :, :, b0:b0 + nb]
                    t96 = work.tile([128, 3, nb], F32, tag=f"t96{sfx}",
                                    name=f"t96{sfx}")
                    te.tensor_add(t96, mrh, kch[:, 0])
                    h96 = work.tile([128, 3, nb], F32, tag=f"h96{sfx}",
                                    name=f"h96{sfx}")
                    nc.scalar.activation(out=h96, in_=t96, func=ACTF.Relu)
                    te.tensor_mul(h96, h96, kch[:, 1])
                    te.tensor_add(h96, h96, kch[:, 2])
                    st["h96"] = h96
                    if warm is not None:
                        nc.tensor.matmul(out=warm[0:16, 0:480],
                                         lhsT=h96[:, 0, 0:16], rhs=kc2d,
                                         start=True, stop=True)

                def s1():
                    h96 = st["h96"]
                    sq96 = work.tile([128, 3, nb], F32, tag=f"sq96{sfx}",
                                     name=f"sq96{sfx}")
                    nc.scalar.activation(out=sq96, in_=h96, func=ACTF.Square)
                    psLN = pstail.tile([1, 4 * W], F32, tag="small",
                                       name=f"psLN{sfx}")
                    nc.tensor.matmul(out=psLN[0:1, 0:W], lhsT=ones[:, 0:1],
                                     rhs=h96, start=True, stop=True)
                    nc.tensor.matmul(out=psLN[0:1, W:2 * W],
                                     lhsT=ones[:, 0:1], rhs=sq96,
                                     start=True, stop=True)
                    st["psLN"] = psLN

                def s2():
                    psLN = st["psLN"]
                    sums = work.tile([1, 2, nb], F32, tag=f"sums{sfx}",
                                     name=f"sums{sfx}")
                    psLNv = psLN[0:1, 0:2 * W].rearrange(
                        "p (x j b) -> p x b j", x=2, j=3)
                    nc.vector.reduce_sum(out=sums[0:1, 0], in_=psLNv[0:1, 0],
                                         axis=AX.X)
                    nc.vector.reduce_sum(out=sums[0:1, 1], in_=psLNv[0:1, 1],
                                         axis=AX.X)
                    muinv = work.tile([1, 2, nb], F32, tag=f"muinv{sfx}",
                                      name=f"muinv{sfx}")
                    nc.vector.tensor_scalar_mul(muinv[0:1, 0], sums[0:1, 0],
                                                1.0 / 384)
                    msq = work.tile([1, nb], F32, tag=f"msq{sfx}",
                                    name=f"msq{sfx}")
                    nc.vector.tensor_mul(msq, muinv[0:1, 0], muinv[0:1, 0])
                    var = work.tile([1, nb], F32, tag=f"var{sfx}",
                                    name=f"var{sfx}")
                    nc.vector.scalar_tensor_tensor(out=var, in0=sums[0:1, 1],
                                                   scalar=1.0 / 384, in1=msq,
                                                   op0=ALU.mult,
                                                   op1=ALU.subtract)
                    nc.vector.tensor_scalar_add(var, var, 1e-5)
                    sd = work.tile([1, nb], F32, tag=f"sd{sfx}",
                                   name=f"sd{sfx}")
                    nc.scalar.activation(out=sd, in_=var, func=ACTF.Sqrt,
                                         bias=0.0)
                    st["sd"] = sd
                    st["muinv"] = muinv

                def s2b():
                    # isolated: waits on ACT Sqrt (+table load); keeping it in
                    # its own stage stops it stalling conv reduces behind it
                    nc.vector.reciprocal(st["muinv"][0:1, 1], st["sd"])

                def s3():
                    psB = pstail.tile([128, 2, nb], F32, tag="small",
                                      name=f"psB{sfx}")
                    nc.tensor.matmul(out=psB, lhsT=ones[0:1, :],
                                     rhs=st["muinv"][0:1], start=True,
                                     stop=True)
                    muinvB = work.tile([128, 2, nb], F32, tag=f"muinvB{sfx}",
                                       name=f"muinvB{sfx}")
                    nc.scalar.copy(out=muinvB, in_=psB)
                    st["muinvB"] = muinvB

                def s4():
                    h96, muinvB = st["h96"], st["muinvB"]
                    kch = kc3[:, :, :, b0:b0 + nb]
                    hn = work.tile([128, 3, nb], F32, tag=f"hn{sfx}",
                                   name=f"hn{sfx}")
                    for j in range(3):
                        te.tensor_sub(hn[:, j], h96[:, j], muinvB[:, 0])
                        te.tensor_mul(hn[:, j], hn[:, j], muinvB[:, 1])
                    te.tensor_mul(hn, hn, kch[:, 3])
                    te.tensor_add(hn, hn, kch[:, 4])
                    st["hn"] = hn
                    if warm is not None:
                        nc.tensor.matmul(out=warm[0:16, 0:480],
                                         lhsT=hn[:, 0, 0:16], rhs=kc2d,
                                         start=True, stop=True)

                def s5():
                    sil = hst["sil"]
                    nc.scalar.activation(out=sil[:, :, q0:q0 + nb],
                                         in_=st["hn"], func=ACTF.Silu)

                def f1():
                    hn2d = st["hn"].rearrange("p j b -> p (j b)")
                    x3 = hn2d.rearrange("p (m w) -> p m w", m=1)
                    m3 = mtab.rearrange("p (m w) -> p m w", w=1)
                    bx, bm = broadcast_tensor_aps(x3, m3)
                    d = work.tile([128, 10, W], F32, tag=f"sp_d{W}",
                                  name=f"sp1{sfx}_d")
                    te.tensor_tensor(out=d, in0=bx, in1=bm, op=ALU.subtract)
                    st["d"] = d

                def f2a():
                    d = st["d"]
                    v = work.tile([128, 10, W], F32, tag=f"sp_v{W}",
                                  name=f"sp1{sfx}_v")
                    nc.scalar.activation(out=v, in_=d, func=ACTF.Relu)
                    v2 = work.tile([128, 10, W], F32, tag=f"sp_v2{W}",
                                   name=f"sp1{sfx}_v2")
                    nc.scalar.activation(out=v2, in_=v, func=ACTF.Square)
                    r = work.tile([128, 10, W], F32, tag=f"sp_r{W}",
                                  name=f"sp1{sfx}_r")
                    te.tensor_mul(r, v2, v)
                    st["r"] = r
                    if warm is not None:
                        nc.tensor.matmul(out=warm[0:16, 0:480],
                                         lhsT=r[:, 0, 0:16], rhs=kc2d,
                                         start=True, stop=True)

                def f2b1():
                    r = st["r"]
                    a = work.tile([128, 6, W], F32, tag=f"sp_s1{W}",
                                  name=f"sp1{sfx}_a")
                    te.tensor_add(a, r[:, 0:6], r[:, 4:10])
                    st["a"] = a

                def f2b2():
                    r = st["r"]
                    bsum = work.tile([128, 6, W], F32, tag=f"sp_s2{W}",
                                     name=f"sp1{sfx}_b")
                    te.tensor_add(bsum, r[:, 1:7], r[:, 3:9])
                    st["bsum"] = bsum

                gs = te is nc.gpsimd
                bs_re = "p g (j b) -> p g j b"

                def f2c1():
                    a, bsum = st["a"], st["bsum"]
                    if warm is not None:
                        nc.tensor.matmul(out=warm[0:16, 0:480],
                                         lhsT=bsum[:, 0, 0:16], rhs=kc2d,
                                         start=True, stop=True)
                    t1 = work.tile([128, 6, W], F32, tag=f"sp_s12{W}",
                                   name=f"sp1{sfx}_t1")
                    if gs:
                        tmp = work.tile([128, 6, W], F32, tag=f"sp_tm{W}",
                                        name=f"sp1{sfx}_tm")
                        nc.gpsimd.tensor_scalar_mul(tmp, bsum, -4.0)
                        nc.gpsimd.tensor_add(t1, tmp, a)
                    else:
                        nc.vector.scalar_tensor_tensor(
                            out=t1, in0=bsum, scalar=-4.0, in1=a,
                            op0=ALU.mult, op1=ALU.add)
                    st["t1"] = t1

                def f2c2():
                    r, t1 = st["r"], st["t1"]
                    bs_view = hst["bs1"][:, :, :, q0:q0 + nb]
                    if gs:
                        tmp = work.tile([128, 6, W], F32, tag=f"sp_tm{W}",
                                        name=f"sp1{sfx}_t2")
                        nc.gpsimd.tensor_scalar_mul(tmp, r[:, 2:8], 6.0)
                        nc.vector.tensor_add(
                            bs_view, tmp.rearrange(bs_re, j=3),
                            t1.rearrange(bs_re, j=3))
                    else:
                        nc.vector.scalar_tensor_tensor(
                            out=bs_view,
                            in0=r[:, 2:8].rearrange(bs_re, j=3),
                            scalar=6.0,
                            in1=t1.rearrange(bs_re, j=3),
                            op0=ALU.mult, op1=ALU.add)

                L, G, P = "lt", "gs", "pe"
                return [(L, s0), (L, s1), (L, s2), (L, s2b), (L, s3),
                        (L, s4), (L, s5), (G, f1), (G, f2a), (G, f2b1),
                        (G, f2b2), (G, f2c1), (G, f2c2)]

            def make_back(hst, sfx, te, warm=None):
                """KAN matmuls + layer-2 for the half described by hst."""
                nb = hst["nb"]
                b0 = hst["b0"]
                st = {}

                def s7a():
                    sil = hst["sil"].rearrange("p j b -> p (j b)")
                    psK1 = pstail.tile([nb, D1OUT], F32, tag="big",
                                       name=f"psK1{sfx}")
                    for j in range(3):
                        nc.tensor.matmul(out=psK1,
                                         lhsT=sil[:, j * nb:(j + 1) * nb],
                                         rhs=w1s[:, j], start=(j == 0),
                                         stop=False)
                    st["psK1"] = psK1

                def mk_s7(j):
                    def s7x():
                        bs1 = hst["bs1"].rearrange("p g j b -> p g (j b)")
                        psK1 = st["psK1"]
                        for g in range(6):
                            nc.tensor.matmul(
                                out=psK1, lhsT=bs1[:, g, j * nb:(j + 1) * nb],
                                rhs=w1s[:, 3 + j * 6 + g],
                                start=False, stop=False)
                    return s7x

                def s8():
                    bs1 = hst["bs1"].rearrange("p g j b -> p g (j b)")
                    psK1 = st["psK1"]
                    for g in range(6):
                        nc.tensor.matmul(
                            out=psK1,
                            lhsT=bs1[:, g, 2 * nb:3 * nb],
                            rhs=w1s[:, 15 + g],
                            start=False, stop=(g == 5))
                    h2s = work.tile([nb, D1OUT], F32, tag=f"h2s{sfx}",
                                    name=f"h2s{sfx}")
                    nc.scalar.copy(out=h2s, in_=psK1)
                    st["h2s"] = h2s

                def s9():
                    h2s = st["h2s"]
                    psT = pstail.tile([128, 4 * nb], F32, tag="big",
                                      name=f"psT{sfx}")
                    for j in range(4):
                        nc.tensor.transpose(out=psT[:, j * nb:(j + 1) * nb],
                                            in_=h2s[:, j * 128:(j + 1) * 128],
                                            identity=id32[0:nb, 0:nb])
                    h2T = work.tile([128, 4 * nb], F32, tag=f"h2T{sfx}",
                                    name=f"h2T{sfx}")
                    nc.scalar.copy(out=h2T, in_=psT)
                    sil2 = work.tile([128, 4 * nb], BF16, tag=f"sil2{sfx}",
                                     name=f"sil2{sfx}")
                    nc.scalar.activation(out=sil2, in_=h2T, func=ACTF.Silu)
                    st["h2T"] = h2T
                    st["sil2"] = sil2

                def g1():
                    W = 4 * nb
                    x3 = st["h2T"].rearrange("p (m w) -> p m w", m=1)
                    m3 = mtab.rearrange("p (m w) -> p m w", w=1)
                    bx, bm = broadcast_tensor_aps(x3, m3)
                    d = work.tile([128, 10, W], F32, tag=f"sp_d{W}",
                                  name=f"sp2{sfx}_d")
                    te.tensor_tensor(out=d, in0=bx, in1=bm, op=ALU.subtract)
                    st["d2"] = d
                    if warm is not None:
                        nc.tensor.matmul(out=warm[0:16, 0:480],
                                         lhsT=d[:, 0, 0:16], rhs=kc2d,
                                         start=True, stop=True)

                def g2a():
                    W = 4 * nb
                    d = st["d2"]
                    v = work.tile([128, 10, W], F32, tag=f"sp_v{W}",
                                  name=f"sp2{sfx}_v")
                    nc.scalar.activation(out=v, in_=d, func=ACTF.Relu)
                    v2 = work.tile([128, 10, W], F32, tag=f"sp_v2{W}",
                                   name=f"sp2{sfx}_v2")
                    nc.scalar.activation(out=v2, in_=v, func=ACTF.Square)
                    r = work.tile([128, 10, W], F32, tag=f"sp_r{W}",
                                  name=f"sp2{sfx}_r")
                    te.tensor_mul(r, v2, v)
                    st["r2"] = r

                def g2b1():
                    W = 4 * nb
                    r = st["r2"]
                    a = work.tile([128, 6, W], F32, tag=f"sp_s1{W}",
                                  name=f"sp2{sfx}_a")
                    te.tensor_add(a, r[:, 0:6], r[:, 4:10])
                    st["a2"] = a
                    if warm is not None:
                        nc.tensor.matmul(out=warm[0:16, 0:480],
                                         lhsT=a[:, 0, 0:16], rhs=kc2d,
                                         start=True, stop=True)

                def g2b2():
                    W = 4 * nb
                    r = st["r2"]
                    bsum = work.tile([128, 6, W], F32, tag=f"sp_s2{W}",
                                     name=f"sp2{sfx}_b")
                    te.tensor_add(bsum, r[:, 1:7], r[:, 3:9])
                    st["bsum2"] = bsum

                gs = te is nc.gpsimd

                def g2c1():
                    W = 4 * nb
                    a, bsum = st["a2"], st["bsum2"]
                    t1 = work.tile([128, 6, W], F32, tag=f"sp_s12{W}",
                                   name=f"sp2{sfx}_t1")
                    if gs:
                        tmp = work.tile([128, 6, W], F32, tag=f"sp_tm{W}",
                                        name=f"sp2{sfx}_tm")
                        nc.gpsimd.tensor_scalar_mul(tmp, bsum, -4.0)
                        nc.gpsimd.tensor_add(t1, tmp, a)
                    else:
                        nc.vector.scalar_tensor_tensor(
                            out=t1, in0=bsum, scalar=-4.0, in1=a,
                            op0=ALU.mult, op1=ALU.add)
                    st["t12"] = t1

                def g2c2():
                    W = 4 * nb
                    r, t1 = st["r2"], st["t12"]
                    bs = work.tile([128, 6, W], BF16, tag=f"sp_bs{W}",
                                   name=f"sp2{sfx}_bs")
                    if gs:
                        tmp = work.tile([128, 6, W], F32, tag=f"sp_tm{W}",
                                        name=f"sp2{sfx}_t2")
                        nc.gpsimd.tensor_scalar_mul(tmp, r[:, 2:8], 6.0)
                        nc.vector.tensor_add(bs, tmp, t1)
                    else:
                        nc.vector.scalar_tensor_tensor(
                            out=bs, in0=r[:, 2:8], scalar=6.0, in1=t1,
                            op0=ALU.mult, op1=ALU.add)
                    st["bs2"] = bs

                def s11a():
                    psK2 = pstail.tile([nb, W2PAD], F32, tag="big",
                                       name=f"psK2{sfx}")
                    for j in range(4):
                        nc.tensor.matmul(out=psK2,
                                         lhsT=st["sil2"][:, j * nb:(j + 1) * nb],
                                         rhs=w2s[:, j], start=(j == 0),
                                         stop=False)
                    st["psK2"] = psK2

                def mk_s11(j):
                    def s11x():
                        psK2 = st["psK2"]
                        for g in range(6):
                            nc.tensor.matmul(
                                out=psK2, lhsT=st["bs2"][:, g, j * nb:(j + 1) * nb],
                                rhs=w2s[:, 4 + j * 6 + g],
                                start=False, stop=False)
                    return s11x

                def s12():
                    psK2 = st["psK2"]
                    for g in range(6):
                        nc.tensor.matmul(
                            out=psK2,
                            lhsT=st["bs2"][:, g, 3 * nb:4 * nb],
                            rhs=w2s[:, 22 + g],
                            start=False, stop=(g == 5))
                    outS = work.tile([nb, D2OUT], F32, tag=f"outS{sfx}",
                                     name=f"outS{sfx}")
                    nc.scalar.copy(out=outS, in_=psK2[:, 0:D2OUT])
                    nc.sync.dma_start(out=out_d[b0:b0 + nb], in_=outS)

                L, G, P = "lt", "gs", "pe"
                return [(P, s7a), (P, mk_s7(0)), (P, mk_s7(1)),
                        (P, s8), (L, s9), (G, g1), (G, g2a),
                        (G, g2b1), (G, g2b2), (G, g2c1), (G, g2c2),
                        (P, s11a), (P, mk_s11(0)), (P, mk_s11(1)),
                        (P, mk_s11(2)), (P, s12)]

            def make_half_tiles(b0, nbh, hx):
                sil = work.tile([128, 3, nbh], BF16, tag=f"silh{hx}",
                                name=f"silh{hx}")
                bs1 = work.tile([128, 6, 3, nbh], BF16, tag=f"bs1h{hx}",
                                name=f"bs1h{hx}")
                return {"b0": b0, "nb": nbh, "sil": sil, "bs1": bs1}

            # ---- conv phase with tail stages interleaved ----
            # half-0 (samples 0-15): LN+spline front and KAN back interleave
            # between conv samples from sample 17
            h0 = make_half_tiles(0, NH, 0)
            h1 = make_half_tiles(NH, NH, 1)
            fstages = make_front(0, NH, h0, "h0", nc.gpsimd)
            bstages = make_back(h0, "h0", nc.gpsimd)
            # back-stage i needs front progress >= bpreq[i] (s7a needs sil
            # from s5; the spline-j matmuls need bs1 complete from f2c2)
            bpreq = [7, 13, 13, 13] + [13] * (len(bstages) - 4)
            fi = bi = 0
            tile_of = {}

            def load(b):
                pt = patches.tile([128, LP], BF16, tag="pt", name=f"pt{b}")
                nc.sync.dma_start(out=pt[0:126], in_=patch_d[:, b, :])
                tile_of[b] = pt

            load(0)
            for gi, (r0, nr, _off, _cj, _f, _l) in enumerate(GROUPS):
                nc.scalar.dma_start(out=wc_tiles[gi][0:nr, :],
                                    in_=wconv[r0:r0 + nr, :])
            nc.scalar.dma_start(out=kc, in_=kconst)
            nc.scalar.dma_start(out=mtab, in_=mtab_d)
            nc.scalar.dma_start(out=id32, in_=id32_d)
            for b in range(1, PREFETCH):
                load(b)

            wz = const.tile([128, 512], BF16, tag="wz", name="wz")
            nc.vector.memset(wz, 0.5)
            pswarm = pstail.tile([16, D1OUT], F32, tag="big", name="pswarm")
            for _ in range(10):
                nc.tensor.matmul(out=pswarm, lhsT=wz[:, 0:16],
                                 rhs=wz, start=True, stop=True)

            last_pt = None
            for b in range(BC):
                if b + PREFETCH < BC:
                    load(b + PREFETCH)
                if b >= 4:
                    for _ in range(5):
                        if wjobs:
                            wt, wd, j = wjobs.pop(0)
                            nc.sync.dma_start(out=wt[:, j], in_=wd[:, j])
                pt = tile_of.pop(b)
                last_pt = pt
                pc = [psconv.tile([128, 1024], F32, tag=f"pc{j}",
                                  name=f"pc{j}") for j in range(3)]
                for gi, (r0, nr, off, cj, first, last) in enumerate(GROUPS):
                    lcj = CONV_L[cj] + (CONV_L[cj] & 1)
                    for (n0, n1) in ((0, 512), (512, lcj)):
                        nc.tensor.matmul(
                            out=pc[cj][:, n0:n1],
                            lhsT=wc_tiles[gi][0:nr, :],
                            rhs=pt[0:nr, off + n0: off + n1],
                            start=first, stop=last,
                        )
                # maxpool: DVE reduces c3 then c2 from PSUM (in emission
                # order, so banks free progressively for sample b+1); conv1
                # drains through ACT to SBUF (frees pc0 without DVE) and a
                # final DVE reduce runs from SBUF.
                c1sb = c1sb_bufs[b % 2]
                nc.vector.reduce_max(out=mraw[:, 2, b:b + 1],
                                     in_=pc[2][:, 0:CONV_L[2]], axis=AX.X)
                nc.scalar.copy(out=c1sb[:, 0:CONV_L[0]],
                               in_=pc[0][:, 0:CONV_L[0]])
                nc.vector.reduce_max(out=mraw[:, 1, b:b + 1],
                                     in_=pc[1][:, 0:CONV_L[1]], axis=AX.X)
                nc.vector.reduce_max(out=mraw[:, 0, b:b + 1],
                                     in_=c1sb[:, 0:CONV_L[0]], axis=AX.X)
                # interleave tail stages: front and back advance as two
                # independent in-order streams, at most one heavy stage per
                # engine class per sample (PE matmul bursts vs GpSimd spline)
                if b >= 16:
                    used = set()
                    taken = 0
                    while taken < 3:
                        prog = False
                        if fi < len(fstages):
                            cls = fstages[fi][0]
                            if cls == "lt" or cls not in used:
                                used.add(cls)
                                fstages[fi][1]()
                                fi += 1
                                taken += 1
                                prog = True
                        if (taken < 3 and bi < len(bstages)
                                and bpreq[bi] <= fi):
                            cls = bstages[bi][0]
                            if cls == "lt" or cls not in used:
                                used.add(cls)
                                bstages[bi][1]()
                                bi += 1
                                taken += 1
                                prog = True
                        if not prog:
                            break
            while fi < len(fstages):
                fstages[fi][1]()
                fi += 1
            while bi < len(bstages):
                bstages[bi][1]()
                bi += 1
            # exposed tail: half-1 front + back; anchored fp32 dummy matmuls
            # (fed by chain intermediates) keep the PE p-state at full clock
            # through the serial elementwise stretches so K1/K2 run warm
            pcw = psconv.tile([128, 1024], F32, tag="pc0", name="pcw")
            for _cls, s in (make_front(NH, NH, h1, "h1", nc.vector, warm=pcw)
                            + make_back(h1, "h1", nc.vector, warm=pcw)):
                s()
    nc.compile()
    return nc


def _host_prep(inputs):
    f = np.float32
    bf = ml_dtypes.bfloat16
    x = np.asarray(inputs["x"], f)
    xT = np.ascontiguousarray(x.transpose(0, 2, 1))  # [B, 21, 1000]
    xTpad = np.zeros((B, C, LP + 5), f)
    xTpad[:, :, :L] = xT
    # full 6-tap patch: patch[s*21+c, b, col] = x[b, c, col+s]
    pat = np.empty((6, C, B, LP), f)
    for s in range(6):
        pat[s] = xTpad[:, :, s:s + LP].transpose(1, 0, 2)
    pat = pat.reshape(126, B, LP).astype(bf)
    patches = [np.ascontiguousarray(pat[:, i * BC:(i + 1) * BC, :])
               for i in range(NCORES)]

    def chunks(w, taps):
        return [np.ascontiguousarray(
            np.asarray(w, f)[:, :, t0:t1].transpose(2, 1, 0).reshape((t1 - t0) * C, 128))
            for t0, t1 in taps]

    wconv = np.concatenate(
        chunks(inputs["conv1_w"], [(0, 4)])
        + chunks(inputs["conv2_w"], [(0, 6), (6, 8)])
        + chunks(inputs["conv3_w"], [(0, 6), (6, 12), (12, 16)]), 0).astype(bf)

    def fold(p):
        g, bb, m, v = (np.asarray(inputs[p + s], f) for s in ("_g", "_b", "_m", "_v"))
        s = g / np.sqrt(v + 1e-5)
        return s, bb - m * s

    s1, t1 = fold("bn1")
    s2, t2 = fold("bn2")
    s3, t3 = fold("bn3")
    s4, t4 = fold("bn4")
    Sall = np.concatenate([s1, s2, s3]) * s4
    Tall = np.concatenate([t1, t2, t3]) * s4 + t4
    cb = np.concatenate([np.asarray(inputs["conv1_b"], f),
                         np.asarray(inputs["conv2_b"], f),
                         np.asarray(inputs["conv3_b"], f)])

    def expand(v):
        return np.repeat(np.asarray(v, f).reshape(3, 128).T[:, :, None], BC, 2)

    kconst = np.stack([expand(cb), expand(Sall), expand(Tall),
                       expand(np.asarray(inputs["ln_g"], f)),
                       expand(np.asarray(inputs["ln_b"], f))], 1)
    kconst = np.ascontiguousarray(kconst.reshape(128, 5, 96))

    # spline scale: bases from relu^3 form come out as (6*3.375)*B_g when
    # v = relu(x - (m-4.5)/1.5); fold 1/(6/3.375) into the spline weights
    spl_scale = 3.375 / 6.0
    bw1 = np.asarray(inputs["base_w1"], f)
    sw1 = np.asarray(inputs["spline_w1"], f) * spl_scale
    w1s = np.empty((128, NW1, D1OUT), f)
    for j in range(3):
        w1s[:, j, :] = bw1[:, j * 128:(j + 1) * 128].T
        for g in range(6):
            w1s[:, 3 + j * 6 + g, :] = sw1[:, j * 128:(j + 1) * 128, g].T
    bw2 = np.asarray(inputs["base_w2"], f)
    sw2 = np.asarray(inputs["spline_w2"], f) * spl_scale
    w2s = np.zeros((128, NW2, W2PAD), f)
    for j in range(4):
        w2s[:, j, :D2OUT] = bw2[:, j * 128:(j + 1) * 128].T
        for g in range(6):
            w2s[:, 4 + j * 6 + g, :D2OUT] = sw2[:, j * 128:(j + 1) * 128, g].T

    mtab = np.tile(((np.arange(10, dtype=f) - 4.5) / 1.5), (128, 1))

    shared = {
        "wconv": np.ascontiguousarray(wconv),
        "kconst": kconst,
        "w1s": np.ascontiguousarray(w1s.astype(bf)),
        "w2s": np.ascontiguousarray(w2s.astype(bf)),
        "mtab": np.ascontiguousarray(mtab),
        "id32": np.eye(32, dtype=f),
    }
    return shared, patches


_NC_CACHE = None


def _get_nc():
    global _NC_CACHE
    if _NC_CACHE is None:
        _NC_CACHE = _build_program()
    return _NC_CACHE


def make_in_maps(inputs):
    shared, patches = _host_prep(inputs)
    return [{**shared, "patch": patches[i]} for i in range(NCORES)]


def kernel(**inputs):
    nc = _get_nc()
    in_maps = make_in_maps(inputs)
    res = run_bass_kernel_spmd(nc, in_maps, list(range(NCORES)))
    return np.concatenate([res.results[i]["out"] for i in range(NCORES)], 0)
